# revision 26
# baseline (speedup 1.0000x reference)
"""Multi-head attention (shared QKV projection, floor-div scores) on 8 NeuronCores.

Problem: B=2, S=2048, HID=1024, NH=16, HD=64
    q = k = v = x @ Wq + bq          (reshaped to heads)
    scores = floor(q k^T / sqrt(64)) ; attn = softmax(scores)
    out = (attn v) @ Wo + bo

Sharding: core c handles batch c//4 and 4 heads ((c%4)*4 ..+4). Each core
computes its heads' contribution to out[b] = attn_out @ Wo; the host sums the
4 partials per batch and adds bo.

Single-pass fp16 device algorithm per core (rel err ~8e-3 vs the 2e-2 gate):
  - host pre-scales xT by 8^-0.5 (and bq by 8^-0.5, Wo by 8^0.5) so the PE
    score matmuls directly produce s8 = scores/sqrt(HD).
  - qT per head stored [65, S] fp16: rows 0-63 = q16, row 64 = 7.96875 const
    whose square (63.5009) rides the score contraction (K=65), implementing
    the +63.5 part of the floor-by-RNE trick for free.
  - v tiles = PE-transposed q16 slices (+ ones column for rowsum Z)
  - J-loop over i-blocks (score psum [128,512], one bank): LAG-deep software
    pipeline so the PE queue never waits on the DVE/ACT chain.
    floor: n = RNE(s8 + 63.5 + 2^23) via a +2^23 add (tensor_single_scalar
    on DVE; Identity+bias on ACT for ACT_Q block positions), staged into
    [128,4096] nt sbuf tiles; one big exp per 8-block group (ACT, fp16 out)
    -> 8x PV matmuls oT[65,512] += v_i^T @ P_i
  - Z normalization: rz = exp(-ln Z) on ACT (same table set as Exp/Identity
    so no table reloads; custom DVE ops are avoided entirely because their
    gpsimd library reload stalls the whole core ~6us), broadcast via DRAM
    round-trip DMA + GPSIMD multiply (last head: PE rank-1 broadcast + DVE
    multiply to shorten the tail-gating chain)
  - partial out per pair: oTn_p^T @ Wo_p fp16 -> part0/part1; host sums the
    8 partials per batch (pair0's outproj fills pair1's pipeline drain)
"""

import math
import sys

sys.path.insert(0, "/opt/trn_rl_repo")

import numpy as np
import concourse.bass as bass
import concourse.bacc as bacc
import concourse.tile as tile
from concourse import mybir
from concourse.bass_utils import run_bass_kernel_spmd

F32 = mybir.dt.float32
F16 = mybir.dt.float16
ADD = mybir.AluOpType.add
MULT = mybir.AluOpType.mult
AF = mybir.ActivationFunctionType

B, S, HID, NH, HD = 2, 2048, 1024, 16, 64
HPC = 4          # heads per core
NCORES = 8
KT = HID // 128  # 8 k-tiles
QT = S // 128    # 16 q/s tiles
C23 = float(2 ** 23)
PSHIFT = 10.0    # P = e^(s_int - PSHIFT); cancels in softmax; keeps P < fp16 max
CROW = 7.96875   # CROW^2 = 63.50098 ~ 63.5 (floor offset, in-contraction)
SQ8 = 1.0 / math.sqrt(8.0)

LAG = 5          # i-blocks the round/exp/PV tail trails the score matmuls


def act_round(h, g, q):
    # which blocks' +2^23 rounds run on ACT (Identity) instead of DVE;
    # tuned so ACT ~ DVE busy in the J-loop
    return q == 3

_NC_CACHE = None


def _build():
    nc = bacc.Bacc("TRN2", target_bir_lowering=False, debug=False,
                   num_devices=NCORES)

    x16d = nc.dram_tensor("x16", [HID, S], F16, kind="ExternalInput")
    wqd, wod, bqrd = [], [], []
    for p in range(2):
        wqd.append(nc.dram_tensor(f"wq{p}", [128, 1024], F16,
                                  kind="ExternalInput"))
        wod.append(nc.dram_tensor(f"wo{p}", [128, 1024], F16,
                                  kind="ExternalInput"))
        bqrd.append(nc.dram_tensor(f"bqr{p}", [128, 1], F32,
                                   kind="ExternalInput"))
    ident = nc.dram_tensor("ident", [128, 64], F16, kind="ExternalInput")
    partd = [nc.dram_tensor(f"part{p}", [S, HID], F16, kind="ExternalOutput")
             for p in range(2)]
    rzscr = nc.dram_tensor("rzscr", [HPC, S], F32)

    with tile.TileContext(nc) as tc:
        with (
            tc.tile_pool(name="cst", bufs=1) as cst,
            tc.tile_pool(name="big", bufs=1) as big,
            tc.tile_pool(name="ppool", bufs=3) as ppool,
            tc.tile_pool(name="ntpool", bufs=3) as ntpool,
            tc.tile_pool(name="osb", bufs=2) as osb,
            tc.tile_pool(name="zs", bufs=1) as zs,
            tc.tile_pool(name="otp", bufs=3) as otp,
            tc.tile_pool(name="ps_np", bufs=LAG + 1, space="PSUM") as ps_np,
            tc.tile_pool(name="ps_ot", bufs=2, space="PSUM") as ps_ot,
        ):
            # ---- constants / inputs ----
            b_negc = cst.tile([128, 1], F32, tag="b_negc")
            nc.vector.memset(b_negc[:], -(C23 + 64.0 + PSHIFT))
            b_c23 = cst.tile([128, 1], F32, tag="b_c23")
            nc.vector.memset(b_c23[:], C23)
            onesr = cst.tile([1, S], F16, tag="onesr")
            nc.vector.memset(onesr[:], 1.0)
            id_t = cst.tile([128, 64], F16, tag="id_t")
            nc.sync.dma_start(id_t[:], ident[:])
            wq_t, wo_t, bqr_t = [], [], []
            for p in range(2):
                w = cst.tile([128, 1024], F16, tag=f"wq_t{p}")
                nc.sync.dma_start(w[:], wqd[p][:])
                wq_t.append(w)
                w = cst.tile([128, 1024], F16, tag=f"wo_t{p}")
                nc.sync.dma_start(w[:], wod[p][:])
                wo_t.append(w)
                w = cst.tile([128, 1], F32, tag=f"bqr_t{p}")
                nc.sync.dma_start(w[:], bqrd[p][:])
                bqr_t.append(w)
            x16_t = []
            for t in range(KT):
                w = big.tile([128, S], F16, tag=f"x16_{t}")
                x16_t.append(w)
            for hf in range(2):
                for t in range(KT):
                    nc.sync.dma_start(
                        x16_t[t][:, hf * 1024:(hf + 1) * 1024],
                        x16d[t * 128:(t + 1) * 128,
                             hf * 1024:(hf + 1) * 1024])

            # qT per head: rows 0-63 = q16, row 64 = CROW const
            q16_t = []
            for h in range(HPC):
                w = big.tile([65, S], F16, tag=f"q16_{h}")
                nc.gpsimd.memset(w[64:65, :], CROW)
                q16_t.append(w)

            # ---- phase 1: qT projection (1-pass fp16) ----
            def emit_proj(p, jhs=(0, 1, 2, 3)):
                # t-outer / jh-inner: consecutive matmuls hit different psum
                # banks, so the accumulation chains pipeline instead of
                # serializing on one bank
                pqs = {jh: ps_np.tile([128, 512], F32, tag="nPps",
                                      name=f"pq{p}_{jh}") for jh in jhs}
                for t in range(KT):
                    for jh in jhs:
                        o = jh * 512
                        nc.tensor.matmul(
                            pqs[jh][:],
                            wq_t[p][:, t * 128:(t + 1) * 128],
                            x16_t[t][:, o:o + 512],
                            start=(t == 0), stop=(t == KT - 1),
                        )
                for jh in jhs:
                    o = jh * 512
                    for hx in range(2):
                        nc.vector.tensor_single_scalar(
                            q16_t[2 * p + hx][0:64, o:o + 512],
                            pqs[jh][hx * 64:hx * 64 + 64, :],
                            bqr_t[p][hx * 64:hx * 64 + 64, :], ADD,
                        )

            # ---- phase 2: v tiles (transposed q16 + ones col) ----
            v_t = [None] * HPC

            def emit_v(h, halves=(0, 1)):
                if v_t[h] is None:
                    vt = big.tile([128, QT * 65], F16, tag=f"v{h}")
                    ones = (vt[:].rearrange("p (t e) -> p t e", e=65)
                            [:, :, 64:65])
                    nc.vector.memset(ones, 1.0)
                    v_t[h] = vt
                vt = v_t[h]
                for half in halves:
                    pv = ps_np.tile([128, 512], F16, tag="nPps",
                                    name=f"pv{h}_{half}")
                    for tt in range(8):
                        i = half * 8 + tt
                        nc.tensor.transpose(
                            pv[:, tt * 64:(tt + 1) * 64],
                            q16_t[h][0:64, i * 128:(i + 1) * 128],
                            id_t[0:64, :],
                        )
                    dst = (vt[:, half * 520:half * 520 + 520]
                           .rearrange("p (t e) -> p t e", e=65)[:, :, 0:64])
                    src = pv[:, 0:512].rearrange("p (t e) -> p t e", e=64)
                    nc.vector.tensor_copy(dst, src)

            # ---- phase 3: attention J-loop, LAG-deep software pipeline ----
            oTn_t = []
            for p in range(2):
                w = big.tile([128, S], F16, tag=f"oTn{p}")
                oTn_t.append(w)

            def emit_pair_attn(p, blocks=None, inject_at=None):
                heads = (2 * p, 2 * p + 1)
                oTsb = {h: osb.tile([65, S], F32, tag="oTsb",
                                    name=f"oTsb{h}") for h in heads}
                poT_t = {}
                nt_t = {}

                def emit_round(h, J, i, nP):
                    g, q = i // 8, i % 8
                    if q == 0:
                        nt_t[(h, J, g)] = ntpool.tile(
                            [128, 4096], F32, tag="ntile",
                            name=f"nt{h}_{J}_{g}")
                    nt = nt_t[(h, J, g)][:, q * 512:(q + 1) * 512]
                    if act_round(h, g, q):
                        nc.scalar.activation(nt, nP[:], AF.Identity,
                                             bias=b_c23[:], scale=1.0)
                    else:
                        nc.vector.tensor_single_scalar(nt, nP[:], C23, ADD)

                def emit_group_tail(h, J, g):
                    jo = J * 512
                    if g == 0:
                        poT_t[(h, J)] = ps_ot.tile(
                            [65, 512], F32, tag="poTps", name=f"poT{h}_{J}")
                    poT = poT_t[(h, J)]
                    nt = nt_t[(h, J, g)]
                    pt = ppool.tile([128, 4096], F16, tag="ptile",
                                    name=f"pt{h}_{J}_{g}")
                    nc.scalar.activation(pt[:], nt[:], AF.Exp,
                                         bias=b_negc[:], scale=1.0)
                    for q in range(8):
                        i = 8 * g + q
                        nc.tensor.matmul(
                            poT[:],
                            v_t[h][:, i * 65:(i + 1) * 65],
                            pt[:, q * 512:(q + 1) * 512],
                            start=(i == 0), stop=(i == QT - 1),
                        )
                    if g == 1:
                        nc.vector.tensor_copy(oTsb[h][:, jo:jo + 512],
                                              poT[:])
                        if J == 3:
                            r = (h % 2) * 64
                            # rz = exp(-ln Z) on ACT: same table set as the
                            # J-loop's Identity/Exp, so no ACT_TABLE_LOAD,
                            # and no custom-DVE op (whose gpsimd lib reload
                            # stalls the whole core ~6us)
                            lnz = zs.tile([1, S], F32, tag="lnz",
                                          name=f"lnz{h}")
                            nc.scalar.activation(lnz[:], oTsb[h][64:65, :],
                                                 AF.Ln, bias=0.0, scale=1.0)
                            rz = zs.tile([1, S], F32, tag="rz",
                                         name=f"rz{h}")
                            nc.scalar.activation(rz[:], lnz[:], AF.Exp,
                                                 bias=0.0, scale=-1.0)
                            if h < HPC - 1:
                                # off critical path: DMA broadcast + gpsimd
                                nc.sync.dma_start(rzscr[h:h + 1, :], rz[:])
                                repz = zs.tile([64, S], F32, tag="repz",
                                               name=f"repz{h}")
                                nc.sync.dma_start(
                                    repz[:],
                                    rzscr[h:h + 1, :].broadcast_to([64, S]))
                                nc.gpsimd.tensor_tensor(
                                    oTn_t[p][r:r + 64, :],
                                    oTsb[h][0:64, :], repz[:], MULT)
                            else:
                                # last head gates the tail: PE broadcast
                                rz16 = zs.tile([1, S], F16, tag="rz16",
                                               name=f"rz16{h}")
                                nc.vector.tensor_copy(rz16[:], rz[:])
                                for j2 in range(4):
                                    o2 = j2 * 512
                                    prz = ps_ot.tile([64, 512], F32,
                                                     tag="poTps",
                                                     name=f"prz{h}_{j2}")
                                    nc.tensor.matmul(
                                        prz[:], onesr[0:1, 0:64],
                                        rz16[:, o2:o2 + 512],
                                        start=True, stop=True,
                                    )
                                    nc.vector.tensor_tensor(
                                        oTn_t[p][r:r + 64, o2:o2 + 512],
                                        prz[:],
                                        oTsb[h][0:64, o2:o2 + 512], MULT)

                def emit_block_tail(h, J, i, nP):
                    emit_round(h, J, i, nP)
                    if i % 8 == 7:
                        emit_group_tail(h, J, i // 8)

                if blocks is None:
                    blks = [(h, J, i) for h in heads for J in range(4)
                            for i in range(QT)]
                else:
                    blks = blocks
                pend = []
                for k, (h, J, i) in enumerate(blks):
                    if inject_at and k in inject_at:
                        inject_at[k]()
                    q16 = q16_t[h]
                    jo = J * 512
                    nP = ps_np.tile([128, 512], F32, tag="nPps",
                                    name=f"nP{h}_{J}_{i}")
                    nc.tensor.matmul(
                        nP[:],
                        q16[0:65, i * 128:(i + 1) * 128],
                        q16[0:65, jo:jo + 512],
                        start=True, stop=True,
                    )
                    pend.append((h, J, i, nP))
                    if len(pend) > LAG:
                        emit_block_tail(*pend.pop(0))
                for args in pend:
                    emit_block_tail(*args)

            def outproj_chunk(p, m):
                # one m-tile of pair p's partial output projection
                def f():
                    ot = otp.tile([128, 1024], F16, tag="otile",
                                  name=f"ot{p}_{m}")
                    for c in range(2):
                        po = ps_np.tile([128, 512], F32, tag="nPps",
                                        name=f"po{p}_{m}_{c}")
                        nc.tensor.matmul(
                            po[:],
                            oTn_t[p][:, m * 128:(m + 1) * 128],
                            wo_t[p][:, c * 512:(c + 1) * 512],
                            start=True, stop=True,
                        )
                        if (m + c) % 2 == 0:
                            nc.vector.tensor_copy(
                                ot[:, c * 512:(c + 1) * 512], po[:])
                        else:
                            nc.scalar.copy(ot[:, c * 512:(c + 1) * 512],
                                           po[:])
                    for hf in range(2):
                        nc.sync.dma_start(
                            partd[p][m * 128:(m + 1) * 128,
                                     hf * 512:(hf + 1) * 512],
                            ot[:, hf * 512:(hf + 1) * 512])
                return f

            # pair0: project/transponse only the first S-half, start the
            # J=0/i<8 blocks (they need nothing else), and emit the second
            # wave while the second half of x is still streaming in
            emit_proj(0, jhs=(0, 1))
            emit_v(0, halves=(0,))
            emit_v(1, halves=(0,))
            p0_blocks = ([(h, 0, i) for h in (0, 1) for i in range(8)]
                         + [(0, 0, i) for i in range(8, 16)]
                         + [(1, 0, i) for i in range(8, 16)]
                         + [(0, J, i) for J in (1, 2, 3) for i in range(16)]
                         + [(1, J, i) for J in (1, 2, 3) for i in range(16)])

            def wave_b():
                emit_proj(0, jhs=(2, 3))
                emit_v(0, halves=(1,))
                emit_v(1, halves=(1,))

            emit_pair_attn(0, blocks=p0_blocks, inject_at={16: wave_b})
            emit_proj(1)
            emit_v(2)
            emit_v(3)
            emit_pair_attn(1)
            # pair0's chunks first: their inputs are long ready, so the PE
            # fills pair1's pipeline-drain window with them
            for m in range(QT):
                outproj_chunk(0, m)()
            for m in range(QT):
                outproj_chunk(1, m)()

    nc.finalize()
    return nc


def _get_nc():
    global _NC_CACHE
    if _NC_CACHE is None:
        _NC_CACHE = _build()
    return _NC_CACHE


def make_in_maps(x, Wq, bq, Wo):
    eye = np.eye(64, dtype=np.float16)
    ident = np.vstack([eye, eye])
    in_maps = []
    for c in range(NCORES):
        b, hb = c // 4, (c % 4) * HPC
        xts = np.ascontiguousarray(x[b].T) * np.float32(SQ8)   # [1024, 2048]
        x16 = xts.astype(np.float16)
        m = {"x16": x16, "ident": ident}
        for p in range(2):
            lo = (hb + 2 * p) * HD          # first col/row of this head pair
            wq_cols = Wq[:, lo:lo + 128]    # [1024, 128]
            # lhsT k-tile layout: [128 part, 8 ktiles x 128]
            wqp = np.ascontiguousarray(
                wq_cols.reshape(KT, 128, 128).transpose(1, 0, 2).reshape(128, 1024)
            )
            m[f"wq{p}"] = wqp.astype(np.float16)
            m[f"wo{p}"] = (np.ascontiguousarray(Wo[lo:lo + 128, :])
                           * np.float32(1.0 / SQ8)).astype(np.float16)
            m[f"bqr{p}"] = (bq[lo:lo + 128, None]
                            * np.float32(SQ8)).astype(np.float32)
        in_maps.append(m)
    return in_maps


def kernel(x, Wq, bq, Wo, bo):
    x = np.asarray(x, np.float32)
    Wq = np.asarray(Wq, np.float32)
    bq = np.asarray(bq, np.float32)
    Wo = np.asarray(Wo, np.float32)
    bo = np.asarray(bo, np.float32)

    in_maps = make_in_maps(x, Wq, bq, Wo)
    res = run_bass_kernel_spmd(_get_nc(), in_maps, list(range(NCORES)))
    out = np.empty((B, S, HID), np.float32)
    for b in range(B):
        acc = np.zeros((S, HID), np.float32)
        for c in range(4 * b, 4 * b + 4):
            acc += res.results[c]["part0"].astype(np.float32)
            acc += res.results[c]["part1"].astype(np.float32)
        out[b] = acc + bo[None, :]
    return out


# revision 27
# speedup vs baseline: 1.0089x; 1.0089x over previous
"""Multi-head attention (shared QKV projection, floor-div scores) on 8 NeuronCores.

Problem: B=2, S=2048, HID=1024, NH=16, HD=64
    q = k = v = x @ Wq + bq          (reshaped to heads)
    scores = floor(q k^T / sqrt(64)) ; attn = softmax(scores)
    out = (attn v) @ Wo + bo

Sharding: core c handles batch c//4 and 4 heads ((c%4)*4 ..+4). Each core
computes its heads' contribution to out[b] = attn_out @ Wo; the host sums the
4 partials per batch and adds bo.

Single-pass fp16 device algorithm per core (rel err ~8e-3 vs the 2e-2 gate):
  - host pre-scales xT by 8^-0.5 (and bq by 8^-0.5, Wo by 8^0.5) so the PE
    score matmuls directly produce s8 = scores/sqrt(HD).
  - qT per head stored [65, S] fp16: rows 0-63 = q16, row 64 = 7.96875 const
    whose square (63.5009) rides the score contraction (K=65), implementing
    the +63.5 part of the floor-by-RNE trick for free.
  - v tiles = PE-transposed q16 slices (+ ones column for rowsum Z)
  - J-loop over i-blocks (score psum [128,512], one bank): LAG-deep software
    pipeline so the PE queue never waits on the DVE/ACT chain.
    floor: n = RNE(s8 + 63.5 + 2^23) via a +2^23 add (tensor_single_scalar
    on DVE; Identity+bias on ACT for ACT_Q block positions), staged into
    [128,4096] nt sbuf tiles; one big exp per 8-block group (ACT, fp16 out)
    -> 8x PV matmuls oT[65,512] += v_i^T @ P_i
  - Z normalization: rz = exp(-ln Z) on ACT (same table set as Exp/Identity
    so no table reloads; custom DVE ops are avoided entirely because their
    gpsimd library reload stalls the whole core ~6us), broadcast via DRAM
    round-trip DMA + GPSIMD multiply (last head: PE rank-1 broadcast + DVE
    multiply to shorten the tail-gating chain)
  - partial out per pair: oTn_p^T @ Wo_p fp16 -> part0/part1; host sums the
    8 partials per batch (pair0's outproj fills pair1's pipeline drain)
"""

import math
import sys

sys.path.insert(0, "/opt/trn_rl_repo")

import numpy as np
import concourse.bass as bass
import concourse.bacc as bacc
import concourse.tile as tile
from concourse import mybir
from concourse.bass_utils import run_bass_kernel_spmd

F32 = mybir.dt.float32
F16 = mybir.dt.float16
ADD = mybir.AluOpType.add
MULT = mybir.AluOpType.mult
AF = mybir.ActivationFunctionType

B, S, HID, NH, HD = 2, 2048, 1024, 16, 64
HPC = 4          # heads per core
NCORES = 8
KT = HID // 128  # 8 k-tiles
QT = S // 128    # 16 q/s tiles
C23 = float(2 ** 23)
PSHIFT = 10.0    # P = e^(s_int - PSHIFT); cancels in softmax; keeps P < fp16 max
CROW = 7.96875   # CROW^2 = 63.50098 ~ 63.5 (floor offset, in-contraction)
SQ8 = 1.0 / math.sqrt(8.0)

LAG = 5          # i-blocks the round/exp/PV tail trails the score matmuls


def act_round(h, g, q):
    # which blocks' +2^23 rounds run on ACT (Identity) instead of DVE;
    # tuned so ACT ~ DVE busy in the J-loop
    return q == 3

_NC_CACHE = None


def _build():
    nc = bacc.Bacc("TRN2", target_bir_lowering=False, debug=False,
                   num_devices=NCORES)

    x16d = nc.dram_tensor("x16", [HID, S], F16, kind="ExternalInput")
    wqd, wod, bqrd = [], [], []
    for p in range(2):
        wqd.append(nc.dram_tensor(f"wq{p}", [128, 1024], F16,
                                  kind="ExternalInput"))
        wod.append(nc.dram_tensor(f"wo{p}", [128, 1024], F16,
                                  kind="ExternalInput"))
        bqrd.append(nc.dram_tensor(f"bqr{p}", [128, 1], F32,
                                   kind="ExternalInput"))
    ident = nc.dram_tensor("ident", [128, 64], F16, kind="ExternalInput")
    partd = [nc.dram_tensor(f"part{p}", [S, HID], F16, kind="ExternalOutput")
             for p in range(2)]
    rzscr = nc.dram_tensor("rzscr", [HPC, S], F32)

    with tile.TileContext(nc) as tc:
        with (
            tc.tile_pool(name="cst", bufs=1) as cst,
            tc.tile_pool(name="big", bufs=1) as big,
            tc.tile_pool(name="ppool", bufs=3) as ppool,
            tc.tile_pool(name="ntpool", bufs=3) as ntpool,
            tc.tile_pool(name="osb", bufs=2) as osb,
            tc.tile_pool(name="zs", bufs=1) as zs,
            tc.tile_pool(name="otp", bufs=3) as otp,
            tc.tile_pool(name="ps_np", bufs=LAG + 1, space="PSUM") as ps_np,
            tc.tile_pool(name="ps_ot", bufs=2, space="PSUM") as ps_ot,
        ):
            # ---- constants / inputs ----
            b_negc = cst.tile([128, 1], F32, tag="b_negc")
            nc.vector.memset(b_negc[:], -(C23 + 64.0 + PSHIFT))
            b_c23 = cst.tile([128, 1], F32, tag="b_c23")
            nc.vector.memset(b_c23[:], C23)
            onesr = cst.tile([1, S], F16, tag="onesr")
            nc.vector.memset(onesr[:], 1.0)
            id_t = cst.tile([128, 64], F16, tag="id_t")
            nc.sync.dma_start(id_t[:], ident[:])
            wq_t, wo_t, bqr_t = [], [], []
            for p in range(2):
                w = cst.tile([128, 1024], F16, tag=f"wq_t{p}")
                nc.sync.dma_start(w[:], wqd[p][:])
                wq_t.append(w)
                w = cst.tile([128, 1024], F16, tag=f"wo_t{p}")
                nc.sync.dma_start(w[:], wod[p][:])
                wo_t.append(w)
                w = cst.tile([128, 1], F32, tag=f"bqr_t{p}")
                nc.sync.dma_start(w[:], bqrd[p][:])
                bqr_t.append(w)
            x16_t = []
            for t in range(KT):
                w = big.tile([128, S], F16, tag=f"x16_{t}")
                for hf in range(2):
                    nc.sync.dma_start(
                        w[:, hf * 1024:(hf + 1) * 1024],
                        x16d[t * 128:(t + 1) * 128,
                             hf * 1024:(hf + 1) * 1024])
                x16_t.append(w)

            # qT per head: rows 0-63 = q16, row 64 = CROW const
            q16_t = []
            for h in range(HPC):
                w = big.tile([65, S], F16, tag=f"q16_{h}")
                nc.gpsimd.memset(w[64:65, :], CROW)
                q16_t.append(w)

            # ---- phase 1: qT projection (1-pass fp16) ----
            def emit_proj(p):
                # t-outer / jh-inner: consecutive matmuls hit different psum
                # banks, so the accumulation chains pipeline instead of
                # serializing on one bank
                pqs = [ps_np.tile([128, 512], F32, tag="nPps",
                                  name=f"pq{p}_{jh}") for jh in range(4)]
                for t in range(KT):
                    for jh in range(4):
                        o = jh * 512
                        nc.tensor.matmul(
                            pqs[jh][:],
                            wq_t[p][:, t * 128:(t + 1) * 128],
                            x16_t[t][:, o:o + 512],
                            start=(t == 0), stop=(t == KT - 1),
                        )
                for jh in range(4):
                    o = jh * 512
                    for hx in range(2):
                        nc.vector.tensor_single_scalar(
                            q16_t[2 * p + hx][0:64, o:o + 512],
                            pqs[jh][hx * 64:hx * 64 + 64, :],
                            bqr_t[p][hx * 64:hx * 64 + 64, :], ADD,
                        )

            # ---- phase 2: v tiles (transposed q16 + ones col) ----
            v_t = [None] * HPC

            def emit_v(h):
                vt = big.tile([128, QT * 65], F16, tag=f"v{h}")
                for half in range(2):
                    pv = ps_np.tile([128, 512], F16, tag="nPps",
                                    name=f"pv{h}_{half}")
                    for tt in range(8):
                        i = half * 8 + tt
                        nc.tensor.transpose(
                            pv[:, tt * 64:(tt + 1) * 64],
                            q16_t[h][0:64, i * 128:(i + 1) * 128],
                            id_t[0:64, :],
                        )
                    dst = (vt[:, half * 520:half * 520 + 520]
                           .rearrange("p (t e) -> p t e", e=65)[:, :, 0:64])
                    src = pv[:, 0:512].rearrange("p (t e) -> p t e", e=64)
                    nc.vector.tensor_copy(dst, src)
                ones = vt[:].rearrange("p (t e) -> p t e", e=65)[:, :, 64:65]
                nc.vector.memset(ones, 1.0)
                v_t[h] = vt

            # ---- phase 3: attention J-loop, LAG-deep software pipeline ----
            oTn_t = []
            for p in range(2):
                w = big.tile([128, S], F16, tag=f"oTn{p}")
                oTn_t.append(w)

            def emit_pair_attn(p, inject=()):
                inject = list(inject)
                heads = (2 * p, 2 * p + 1)
                oTsb = {h: osb.tile([65, S], F32, tag="oTsb",
                                    name=f"oTsb{h}") for h in heads}
                poT_t = {}
                nt_t = {}

                def emit_round(h, J, i, nP):
                    g, q = i // 8, i % 8
                    if q == 0:
                        nt_t[(h, J, g)] = ntpool.tile(
                            [128, 4096], F32, tag="ntile",
                            name=f"nt{h}_{J}_{g}")
                    nt = nt_t[(h, J, g)][:, q * 512:(q + 1) * 512]
                    if act_round(h, g, q):
                        nc.scalar.activation(nt, nP[:], AF.Identity,
                                             bias=b_c23[:], scale=1.0)
                    else:
                        nc.vector.tensor_single_scalar(nt, nP[:], C23, ADD)

                def emit_group_tail(h, J, g):
                    jo = J * 512
                    if g == 0:
                        poT_t[(h, J)] = ps_ot.tile(
                            [65, 512], F32, tag="poTps", name=f"poT{h}_{J}")
                    poT = poT_t[(h, J)]
                    nt = nt_t[(h, J, g)]
                    pt = ppool.tile([128, 4096], F16, tag="ptile",
                                    name=f"pt{h}_{J}_{g}")
                    nc.scalar.activation(pt[:], nt[:], AF.Exp,
                                         bias=b_negc[:], scale=1.0)
                    for q in range(8):
                        i = 8 * g + q
                        nc.tensor.matmul(
                            poT[:],
                            v_t[h][:, i * 65:(i + 1) * 65],
                            pt[:, q * 512:(q + 1) * 512],
                            start=(i == 0), stop=(i == QT - 1),
                        )
                    if g == 1:
                        nc.vector.tensor_copy(oTsb[h][:, jo:jo + 512],
                                              poT[:])
                        if J == 3:
                            r = (h % 2) * 64
                            # rz = exp(-ln Z) on ACT: same table set as the
                            # J-loop's Identity/Exp, so no ACT_TABLE_LOAD,
                            # and no custom-DVE op (whose gpsimd lib reload
                            # stalls the whole core ~6us)
                            lnz = zs.tile([1, S], F32, tag="lnz",
                                          name=f"lnz{h}")
                            nc.scalar.activation(lnz[:], oTsb[h][64:65, :],
                                                 AF.Ln, bias=0.0, scale=1.0)
                            rz = zs.tile([1, S], F32, tag="rz",
                                         name=f"rz{h}")
                            nc.scalar.activation(rz[:], lnz[:], AF.Exp,
                                                 bias=0.0, scale=-1.0)
                            if h < HPC - 1:
                                # off critical path: DMA broadcast + gpsimd
                                nc.sync.dma_start(rzscr[h:h + 1, :], rz[:])
                                repz = zs.tile([64, S], F32, tag="repz",
                                               name=f"repz{h}")
                                nc.sync.dma_start(
                                    repz[:],
                                    rzscr[h:h + 1, :].broadcast_to([64, S]))
                                nc.gpsimd.tensor_tensor(
                                    oTn_t[p][r:r + 64, :],
                                    oTsb[h][0:64, :], repz[:], MULT)
                            else:
                                # last head gates the tail: PE broadcast
                                rz16 = zs.tile([1, S], F16, tag="rz16",
                                               name=f"rz16{h}")
                                nc.vector.tensor_copy(rz16[:], rz[:])
                                for j2 in range(4):
                                    o2 = j2 * 512
                                    prz = ps_ot.tile([64, 512], F32,
                                                     tag="poTps",
                                                     name=f"prz{h}_{j2}")
                                    nc.tensor.matmul(
                                        prz[:], onesr[0:1, 0:64],
                                        rz16[:, o2:o2 + 512],
                                        start=True, stop=True,
                                    )
                                    nc.vector.tensor_tensor(
                                        oTn_t[p][r:r + 64, o2:o2 + 512],
                                        prz[:],
                                        oTsb[h][0:64, o2:o2 + 512], MULT)

                def emit_block_tail(h, J, i, nP):
                    emit_round(h, J, i, nP)
                    if i % 8 == 7:
                        emit_group_tail(h, J, i // 8)

                pend = []
                for h in heads:
                    q16 = q16_t[h]
                    for J in range(4):
                        jo = J * 512
                        for i in range(QT):
                            nP = ps_np.tile([128, 512], F32, tag="nPps",
                                            name=f"nP{h}_{J}_{i}")
                            nc.tensor.matmul(
                                nP[:],
                                q16[0:65, i * 128:(i + 1) * 128],
                                q16[0:65, jo:jo + 512],
                                start=True, stop=True,
                            )
                            pend.append((h, J, i, nP))
                            if len(pend) > LAG:
                                emit_block_tail(*pend.pop(0))
                            blk = (h % 2) * 64 + J * QT + i
                            if inject and blk >= 12 and blk % 6 == 0:
                                inject.pop(0)()
                for args in pend:
                    emit_block_tail(*args)
                for f in inject:
                    f()

            def outproj_chunk(p, m):
                # one m-tile of pair p's partial output projection
                def f():
                    ot = otp.tile([128, 1024], F16, tag="otile",
                                  name=f"ot{p}_{m}")
                    for c in range(2):
                        po = ps_np.tile([128, 512], F32, tag="nPps",
                                        name=f"po{p}_{m}_{c}")
                        nc.tensor.matmul(
                            po[:],
                            oTn_t[p][:, m * 128:(m + 1) * 128],
                            wo_t[p][:, c * 512:(c + 1) * 512],
                            start=True, stop=True,
                        )
                        if (m + c) % 2 == 0:
                            nc.vector.tensor_copy(
                                ot[:, c * 512:(c + 1) * 512], po[:])
                        else:
                            nc.scalar.copy(ot[:, c * 512:(c + 1) * 512],
                                           po[:])
                    for hf in range(2):
                        nc.sync.dma_start(
                            partd[p][m * 128:(m + 1) * 128,
                                     hf * 512:(hf + 1) * 512],
                            ot[:, hf * 512:(hf + 1) * 512])
                return f

            emit_proj(0)
            emit_v(0)
            emit_v(1)
            emit_pair_attn(0)
            emit_proj(1)
            emit_v(2)
            emit_v(3)
            emit_pair_attn(1)
            # pair0's chunks first: their inputs are long ready, so the PE
            # fills pair1's pipeline-drain window with them
            for m in range(QT):
                outproj_chunk(0, m)()
            for m in range(QT):
                outproj_chunk(1, m)()

    nc.finalize()
    return nc


def _get_nc():
    global _NC_CACHE
    if _NC_CACHE is None:
        _NC_CACHE = _build()
    return _NC_CACHE


def make_in_maps(x, Wq, bq, Wo):
    eye = np.eye(64, dtype=np.float16)
    ident = np.vstack([eye, eye])
    in_maps = []
    for c in range(NCORES):
        b, hb = c // 4, (c % 4) * HPC
        xts = np.ascontiguousarray(x[b].T) * np.float32(SQ8)   # [1024, 2048]
        x16 = xts.astype(np.float16)
        m = {"x16": x16, "ident": ident}
        for p in range(2):
            lo = (hb + 2 * p) * HD          # first col/row of this head pair
            wq_cols = Wq[:, lo:lo + 128]    # [1024, 128]
            # lhsT k-tile layout: [128 part, 8 ktiles x 128]
            wqp = np.ascontiguousarray(
                wq_cols.reshape(KT, 128, 128).transpose(1, 0, 2).reshape(128, 1024)
            )
            m[f"wq{p}"] = wqp.astype(np.float16)
            m[f"wo{p}"] = (np.ascontiguousarray(Wo[lo:lo + 128, :])
                           * np.float32(1.0 / SQ8)).astype(np.float16)
            m[f"bqr{p}"] = (bq[lo:lo + 128, None]
                            * np.float32(SQ8)).astype(np.float32)
        in_maps.append(m)
    return in_maps


def kernel(x, Wq, bq, Wo, bo):
    x = np.asarray(x, np.float32)
    Wq = np.asarray(Wq, np.float32)
    bq = np.asarray(bq, np.float32)
    Wo = np.asarray(Wo, np.float32)
    bo = np.asarray(bo, np.float32)

    in_maps = make_in_maps(x, Wq, bq, Wo)
    res = run_bass_kernel_spmd(_get_nc(), in_maps, list(range(NCORES)))
    out = np.empty((B, S, HID), np.float32)
    for b in range(B):
        acc = np.zeros((S, HID), np.float32)
        for c in range(4 * b, 4 * b + 4):
            acc += res.results[c]["part0"].astype(np.float32)
            acc += res.results[c]["part1"].astype(np.float32)
        out[b] = acc + bo[None, :]
    return out
